# revision 3
# baseline (speedup 1.0000x reference)
"""MoE ResNet kernel for 8 Trainium2 NeuronCores.

Strategy: data-parallel over the batch (4096 tokens -> 512 per core), all
weights replicated, no collectives.  Activations live feature-major in SBUF
([feature(partitions), token(free)]) so every weight matrix is consumed in its
natural [in, out] layout as the matmul's stationary operand; obs is transposed
on the host while sharding.  The routing path (encoder ResNet + gate logits +
top-2 softmax) runs in fp32 so the top-k expert selection matches the fp32
reference; the expert and value ResNets run in bf16 with fp32 PSUM
accumulation.  Per-expert outputs are combined token-major with the sparse
gate weights, which zero out non-dispatched (token, expert) pairs exactly as
the reference's dense-einsum-with-zero-gates does.
"""

import sys

sys.path.insert(0, "/opt/trn_rl_repo")

import numpy as np
import ml_dtypes

import concourse.bass as bass
import concourse.tile as tile
from concourse import mybir
from concourse.bass import ts
from concourse.bass_utils import run_bass_kernel_spmd

F32 = mybir.dt.float32
BF16 = mybir.dt.bfloat16
BF16_NP = ml_dtypes.bfloat16

B, OBS, HID, LAT, OUT, E, NB = 4096, 1024, 1024, 512, 19, 8, 3
NCORES = 8
T = B // NCORES  # tokens per core
P = 128
TM = T // P  # token m-tiles (4)
EPS = float(np.finfo(np.float64).eps)
NEG_BIG = -1.0e30


def _split_sync_waits(nc):
    """This container's walrus accepts only one sync wait per instruction;
    Tile attaches one wait per producer processor.  Hoist all but the last
    wait of each instruction onto fresh NoOps inserted right before it on the
    same engine (engines run their stream in order, so this is equivalent)."""
    ctr = 0
    for f in nc.m.functions:
        for bb in f.blocks:
            out = []
            changed = False
            for inst in bb.instructions:
                si = inst.sync_info
                if si is not None and si.on_wait and len(si.on_wait) > 1:
                    waits = list(si.on_wait)
                    for w in waits[:-1]:
                        ctr += 1
                        nop = mybir.InstNoOp(name=f"waitsplit-{ctr}", ins=[], outs=[])
                        nop.engine = inst.engine
                        nop.sync_info = mybir.SyncInfo(on_wait=[w], on_update=[])
                        out.append(nop)
                    si.on_wait = waits[-1:]
                    changed = True
                out.append(inst)
            if changed:
                bb.instructions = out
    return ctr


def build_bass():
    nc = bass.Bass()

    def din(name, shape, dt=F32):
        return nc.dram_tensor(name, list(shape), dt, kind="ExternalInput")

    obs_t = din("obs_t", [OBS, T])                 # fp32, transposed on host
    obs_tb = din("obs_tb", [OBS, T], BF16)         # bf16 copy for value net
    enc_W1 = din("enc_W1", [NB, OBS, HID])
    enc_b1 = din("enc_b1", [NB, HID])
    enc_W2 = din("enc_W2", [NB, HID, OBS])
    enc_b2 = din("enc_b2", [NB, OBS])
    enc_Wf = din("enc_Wf", [OBS, LAT])
    enc_bf = din("enc_bf", [LAT])
    w_gate = din("w_gate", [LAT, E])
    exp_W1 = din("exp_W1", [E, NB, LAT, HID], BF16)
    exp_b1 = din("exp_b1", [E, NB, HID])
    exp_W2 = din("exp_W2", [E, NB, HID, LAT], BF16)
    exp_b2 = din("exp_b2", [E, NB, LAT])
    exp_Wf = din("exp_Wf", [E, LAT, OUT], BF16)
    exp_bf_rep = din("exp_bf_rep", [E, P, OUT])    # bias replicated across partitions
    val_W1 = din("val_W1", [NB, OBS, HID], BF16)
    val_b1 = din("val_b1", [NB, HID])
    val_W2 = din("val_W2", [NB, HID, OBS], BF16)
    val_b2 = din("val_b2", [NB, OBS])
    val_Wf = din("val_Wf", [OBS, 1], BF16)
    val_bf_rep = din("val_bf_rep", [P, 1])
    out = nc.dram_tensor("out", [T, OUT + 1], F32, kind="ExternalOutput")

    with tile.TileContext(nc) as tc:
        with (
            tc.tile_pool(name="persist", bufs=1) as persist,
            tc.tile_pool(name="acts", bufs=1) as acts,
            tc.tile_pool(name="res", bufs=2) as res,
            tc.tile_pool(name="wts", bufs=4) as wts,
            tc.tile_pool(name="bias", bufs=2) as bias_pool,
            tc.tile_pool(name="small", bufs=4) as small,
            tc.tile_pool(name="psb", bufs=4, space="PSUM") as psb,
            tc.tile_pool(name="pss", bufs=2, space="PSUM") as pss,
        ):
            # ---------------- load activations ----------------
            x_f32 = persist.tile([P, OBS // P, T], F32)
            nc.sync.dma_start(x_f32[:], obs_t.rearrange("(ko p) t -> p ko t", p=P))
            x_bf = persist.tile([P, OBS // P, T], BF16)
            nc.sync.dma_start(x_bf[:], obs_tb.rearrange("(ko p) t -> p ko t", p=P))

            def load_bias(b_dram, mo, name):
                b_sb = bias_pool.tile([P, mo], F32, name=name, tag=f"b{mo}")
                nc.sync.dma_start(b_sb[:], b_dram.rearrange("(m p) -> p m", p=P))
                return b_sb

            def layer(src, W, b_dram, dt, wtag, act, res_src=None, out_dt=None):
                """dst[feat_out, tok] = act(src @ W + b) (+ res_src).
                src: [P, KO, T]; W dram [KIN, MOUT]."""
                kin, mout = W.shape
                ko, mo = kin // P, mout // P
                wv = W.rearrange("(ko p) m -> p ko m", p=P)
                b_sb = load_bias(b_dram, mo, f"b_{wtag}")
                # ResNet block outputs feed the next block's reads while the
                # following block output is being written -> need 2 slots.
                dst_pool = acts if act == "relu" else res
                dst = dst_pool.tile(
                    [P, mo, T], out_dt or dt, name=f"h_{wtag}", tag=f"h_{wtag}"
                )
                for m in range(mo):
                    wcol = wts.tile([P, ko, P], dt, name=f"w_{wtag}", tag=wtag)
                    nc.sync.dma_start(wcol[:], wv[:, :, ts(m, P)])
                    ps = psb.tile([P, T], F32, name="ps", tag="ps")
                    for k in range(ko):
                        nc.tensor.matmul(
                            ps[:], wcol[:, k, :], src[:, k, :],
                            start=(k == 0), stop=(k == ko - 1),
                        )
                    if act == "relu":
                        nc.scalar.activation(
                            dst[:, m, :], ps[:],
                            mybir.ActivationFunctionType.Relu,
                            bias=b_sb[:, m : m + 1],
                        )
                    else:  # bias add (+ residual)
                        nc.scalar.activation(
                            dst[:, m, :], ps[:],
                            mybir.ActivationFunctionType.Identity,
                            bias=b_sb[:, m : m + 1],
                        )
                        if res_src is not None:
                            nc.vector.tensor_add(
                                dst[:, m, :], dst[:, m, :], res_src[:, m, :]
                            )
                return dst

            def resnet(x0, W1, b1, W2, b2, dt, tag):
                h = x0
                for nb in range(NB):
                    hid = layer(h, W1[nb], b1[nb], dt, f"{tag}w1", "relu")
                    h = layer(
                        hid, W2[nb], b2[nb], dt, f"{tag}w2", "bias",
                        res_src=h,
                        out_dt=dt,
                    )
                return h

            # ---------------- encoder (fp32: feeds routing) ----------------
            h_enc = resnet(x_f32, enc_W1, enc_b1, enc_W2, enc_b2, F32, "enc")

            # latent [P, LAT//P, T] fp32 + bf16 copy
            latent = persist.tile([P, LAT // P, T], F32)
            latent_bf = persist.tile([P, LAT // P, T], BF16)
            bf_sb = load_bias(enc_bf, LAT // P, "b_encf")
            wfv = enc_Wf.rearrange("(ko p) m -> p ko m", p=P)
            for m in range(LAT // P):
                wcol = wts.tile([P, OBS // P, P], F32, name="w_encf", tag="encw1")
                nc.sync.dma_start(wcol[:], wfv[:, :, ts(m, P)])
                ps = psb.tile([P, T], F32, name="ps", tag="ps")
                for k in range(OBS // P):
                    nc.tensor.matmul(
                        ps[:], wcol[:, k, :], h_enc[:, k, :],
                        start=(k == 0), stop=(k == OBS // P - 1),
                    )
                nc.scalar.activation(
                    latent[:, m, :], ps[:],
                    mybir.ActivationFunctionType.Identity,
                    bias=bf_sb[:, m : m + 1],
                )
                nc.vector.tensor_copy(latent_bf[:, m, :], latent[:, m, :])

            # ---------------- gate logits + top-2 softmax ----------------
            wg_sb = persist.tile([P, LAT // P, E], F32)
            nc.sync.dma_start(wg_sb[:], w_gate.rearrange("(ko p) e -> p ko e", p=P))
            logits = persist.tile([P, TM, E], F32)
            for mt in range(TM):
                psg = pss.tile([P, E], F32, name="psg", tag="psg", bufs=1)
                for k in range(LAT // P):
                    nc.tensor.matmul(
                        psg[:], latent[:, k, ts(mt, P)], wg_sb[:, k, :],
                        start=(k == 0), stop=(k == LAT // P - 1),
                    )
                nc.scalar.copy(logits[:, mt, :], psg[:])

            def bc(ap):  # [P, TM] -> broadcast [P, TM, E]
                return ap[:, :, None].to_broadcast((P, TM, E))

            v1 = small.tile([P, TM], F32, name="v1")
            nc.vector.reduce_max(v1[:], logits[:], axis=mybir.AxisListType.X)
            eqm = small.tile([P, TM, E], F32, name="eqm")
            nc.vector.tensor_tensor(
                eqm[:], logits[:], bc(v1), op=mybir.AluOpType.is_equal
            )
            nc.vector.tensor_scalar_mul(eqm[:], eqm[:], NEG_BIG)
            masked = small.tile([P, TM, E], F32, name="masked")
            nc.vector.tensor_add(masked[:], logits[:], eqm[:])
            v2 = small.tile([P, TM], F32, name="v2")
            nc.vector.reduce_max(v2[:], masked[:], axis=mybir.AxisListType.X)
            mask2 = small.tile([P, TM, E], F32, name="mask2")
            nc.vector.tensor_tensor(
                mask2[:], logits[:], bc(v2), op=mybir.AluOpType.is_ge
            )
            dlog = small.tile([P, TM, E], F32, name="dlog")
            nc.vector.tensor_tensor(
                dlog[:], logits[:], bc(v1), op=mybir.AluOpType.subtract
            )
            expd = small.tile([P, TM, E], F32, name="expd")
            nc.scalar.activation(expd[:], dlog[:], mybir.ActivationFunctionType.Exp)
            e2 = small.tile([P, TM, E], F32, name="e2")
            nc.vector.tensor_mul(e2[:], expd[:], mask2[:])
            ssum = small.tile([P, TM], F32, name="ssum")
            nc.vector.reduce_sum(ssum[:], e2[:], axis=mybir.AxisListType.X)
            rsum = small.tile([P, TM], F32, name="rsum")
            nc.vector.reciprocal(rsum[:], ssum[:])
            gates = persist.tile([P, TM, E], F32)
            nc.vector.tensor_tensor(
                gates[:], e2[:], bc(rsum), op=mybir.AluOpType.mult
            )

            # ---------------- output accumulator ----------------
            yv = persist.tile([P, TM, OUT + 1], F32)
            nc.gpsimd.memset(yv[:], 0.0)

            # ---------------- experts (bf16, dense) ----------------
            ebf_sb = persist.tile([P, E, OUT], F32)
            nc.sync.dma_start(ebf_sb[:], exp_bf_rep.rearrange("e p f -> p e f"))
            for e in range(E):
                h = resnet(
                    latent_bf, exp_W1[e], exp_b1[e], exp_W2[e], exp_b2[e],
                    BF16, "exp",
                )
                wf_sb = small.tile([P, LAT // P, OUT], BF16, name="wf_sb", tag="wf")
                nc.sync.dma_start(
                    wf_sb[:], exp_Wf[e].rearrange("(ko p) f -> p ko f", p=P)
                )
                for mt in range(TM):
                    psf = pss.tile([P, OUT], F32, name="psf", tag="psf")
                    for k in range(LAT // P):
                        nc.tensor.matmul(
                            psf[:], h[:, k, ts(mt, P)], wf_sb[:, k, :],
                            start=(k == 0), stop=(k == LAT // P - 1),
                        )
                    eo = small.tile([P, OUT], F32, name="eo", tag="eo")
                    nc.vector.tensor_add(eo[:], psf[:], ebf_sb[:, e, :])
                    geo = small.tile([P, OUT], F32, name="geo", tag="geo")
                    nc.vector.tensor_scalar_mul(
                        geo[:], eo[:], gates[:, mt, e : e + 1]
                    )
                    nc.vector.tensor_add(
                        yv[:, mt, :OUT], yv[:, mt, :OUT], geo[:]
                    )

            # SparseDispatcher.combine quirk: exact zeros -> EPS
            eq0 = small.tile([P, TM, OUT], F32, name="eq0")
            nc.vector.tensor_scalar(
                eq0[:], yv[:, :, :OUT], 0.0, EPS,
                op0=mybir.AluOpType.is_equal, op1=mybir.AluOpType.mult,
            )
            nc.vector.tensor_add(yv[:, :, :OUT], yv[:, :, :OUT], eq0[:])

            # ---------------- value resnet (bf16) ----------------
            h_val = resnet(x_bf, val_W1, val_b1, val_W2, val_b2, BF16, "val")
            vwf_sb = small.tile([P, OBS // P, 1], BF16, name="vwf_sb")
            nc.sync.dma_start(
                vwf_sb[:], val_Wf.rearrange("(ko p) f -> p ko f", p=P)
            )
            vbf_sb = small.tile([P, 1], F32, name="vbf_sb")
            nc.sync.dma_start(vbf_sb[:], val_bf_rep[:, :])
            for mt in range(TM):
                psv = pss.tile([P, 1], F32, name="psv", tag="psv", bufs=1)
                for k in range(OBS // P):
                    nc.tensor.matmul(
                        psv[:], h_val[:, k, ts(mt, P)], vwf_sb[:, k, :],
                        start=(k == 0), stop=(k == OBS // P - 1),
                    )
                nc.scalar.activation(
                    yv[:, mt, OUT : OUT + 1], psv[:],
                    mybir.ActivationFunctionType.Identity,
                    bias=vbf_sb[:, :],
                )

            # ---------------- store ----------------
            nc.sync.dma_start(
                out.rearrange("(mt p) f -> p mt f", p=P), yv[:]
            )

    _split_sync_waits(nc)
    return nc


_CACHE = {}


def _prep_inputs(inputs):
    f32 = np.float32

    def a(name, dt=f32):
        return np.ascontiguousarray(np.asarray(inputs[name], dtype=dt))

    obs = a("obs")
    shared = {
        "enc_W1": a("enc_W1"), "enc_b1": a("enc_b1"),
        "enc_W2": a("enc_W2"), "enc_b2": a("enc_b2"),
        "enc_Wf": a("enc_Wf"), "enc_bf": a("enc_bf"),
        "w_gate": a("w_gate"),
        "exp_W1": a("exp_W1", BF16_NP), "exp_b1": a("exp_b1"),
        "exp_W2": a("exp_W2", BF16_NP), "exp_b2": a("exp_b2"),
        "exp_Wf": a("exp_Wf", BF16_NP),
        "exp_bf_rep": np.ascontiguousarray(
            np.broadcast_to(
                np.asarray(inputs["exp_bf"], dtype=f32)[:, None, :], (E, P, OUT)
            )
        ),
        "val_W1": a("val_W1", BF16_NP), "val_b1": a("val_b1"),
        "val_W2": a("val_W2", BF16_NP), "val_b2": a("val_b2"),
        "val_Wf": a("val_Wf", BF16_NP),
        "val_bf_rep": np.ascontiguousarray(
            np.broadcast_to(np.asarray(inputs["val_bf"], dtype=f32)[None, :], (P, 1))
        ),
    }
    in_maps = []
    for c in range(NCORES):
        shard = np.ascontiguousarray(obs[c * T : (c + 1) * T].T)
        m = dict(shared)
        m["obs_t"] = shard
        m["obs_tb"] = shard.astype(BF16_NP)
        in_maps.append(m)
    return in_maps


def kernel(**inputs) -> np.ndarray:
    if "nc" not in _CACHE:
        _CACHE["nc"] = build_bass()
    nc = _CACHE["nc"]
    in_maps = _prep_inputs(inputs)
    res = run_bass_kernel_spmd(nc, in_maps, core_ids=list(range(NCORES)))
    return np.concatenate([res.results[c]["out"] for c in range(NCORES)], axis=0)
